# revision 12
# baseline (speedup 1.0000x reference)
"""Trainium2 Bass kernel for a 2-layer CIN (Compressed Interaction Network).

Reference computation (per batch b, embedding dim d):
    h1[q] = sum_{f,g} x[f] x[g] W0[q, f*39+g]          (f,g in 0..38)
    h2[h] = sum_{f,q} x[f] h1[q] W1[h, f*128+q]        (f in 0..38, q in 0..127)
    out[b] = concat(sum_d h1, sum_d h2)                 -> [B, 256]

Device mapping (data-parallel over batch across 8 cores, 256 b's each):
  * Layer 1 uses a polarization ("sum of squares") identity so the outer
    product x (x) x never materializes:  x_i x_j = ((x_i+x_j)^2 - x_i^2 - x_j^2)/2.
    With 780 fixed linear forms V (39 singles + 741 pair sums, padded to
    896 = 7*128) and re-packed coefficients C:  h1 = C^T (V^T x)^2.
  * Layer 2 exploits  sum_d h2[b,:,d] = W1flat @ vec(S_b),
    S_b[f,q] = sum_d x[b,f,d] h1[b,q,d]:  h1 is transposed on the PE in
    [8 batches x 16 d] = 128-partition blocks, then S^T chunks come from
    k=128 matmuls against a host-packed block-diagonal transposed-x
    operand (8 batches, 8*40 cols; the 40th column of each batch block is
    all-ones over d, so the same matmul also emits sum_d h1 = out1 free).
    Final contraction: 2 x 39 k=128 matmuls against W1.

Pipeline: rounds r with a 4-deep software pipeline; per round the PE runs
V(r) interleaved with C(r-1), then T(r-2) transposes, then S(r-3) chunk
matmuls; squares split across ACT(even)/DVE(odd); ACT also copies
h1->SBUF and h1^T->SBUF; DVE copies S^T chunks into the sall buffer.
"""

import numpy as np

import concourse.mybir as mybir
import concourse.tile as tile
from concourse import bacc
from concourse.bass import ts
from concourse.bass_utils import run_bass_kernel_spmd

B, F0, D = 2048, 39, 16
H1, H2 = 128, 128
NCORES = 8
BC = B // NCORES          # 256 batches per core
BT = 32                   # batches per tile (round)
NT = BC // BT             # 8 tiles per core
N = BT * D                # 512 columns per tile (cols = (b, d), d inner)
NFP = 896                 # forms padded to 7*128
NCHUNK = 7
CW = 128                  # forms per chunk
SB = 8                    # batches per S-chunk (8 b x 16 d = 128 partitions)
NSC = BT // SB            # 4 S-chunks per tile
FW = F0 + 1               # 39 f cols + 1 ones col (-> sum_d h1)
SW = SB * FW              # 320 cols per S-chunk

F16 = mybir.dt.float16
F32 = mybir.dt.float32


def pack_weights(W0: np.ndarray, W1: np.ndarray):
    """Host-side repack of CIN weights into device layouts (fp16)."""
    W0m = W0[:, :, 0].reshape(H1, F0, F0).astype(np.float64)
    W1m = W1[:, :, 0].reshape(H2, F0, H1).astype(np.float64)

    V = np.zeros((128, NFP), dtype=np.float64)   # k-padded: rows 39.. = 0
    C = np.zeros((NFP, H1), dtype=np.float64)
    for i in range(F0):
        V[i, i] = 1.0
        Bi = W0m[:, i, :] + W0m[:, :, i]          # [H, F]
        C[i, :] = W0m[:, i, i] - 0.5 * (Bi.sum(axis=1) - 2.0 * W0m[:, i, i])
    k = F0
    for i in range(F0):
        for j in range(i + 1, F0):
            V[i, k] = 1.0
            V[j, k] = 1.0
            C[k, :] = 0.5 * (W0m[:, i, j] + W0m[:, j, i])
            k += 1
    c_pack = C.reshape(NCHUNK, CW, H1).transpose(1, 0, 2)   # [128, 7, 128]

    w1p = W1m.transpose(2, 1, 0)                   # [q=128, f=39, h=128]

    ident = np.eye(128, dtype=np.float16)

    return {
        "vp": np.ascontiguousarray(V, dtype=np.float16),
        "cp": np.ascontiguousarray(c_pack, dtype=np.float16),
        "w1p": np.ascontiguousarray(w1p, dtype=np.float16),
        "ident": ident,
    }


def pack_x(x_core: np.ndarray):
    """Per-core input repack: f-major padded x + block-diagonal transposed x.

    x_core: [BC, 39, 16] float.  Returns
      xp  [128, BC, D] fp16, xp[f, b, d] = x[b, f, d] (rows 39.. zero);
      xt3 [NT, NSC, 128, 320] fp16: chunk (t, c) covers batches 32t+8c+j,
          partition 16j+d, col 40j+f, value x[b, f, d]; col 40j+39 is 1.0
          on partitions 16j..16j+15 (emits sum_d h1 through the S matmul).
    """
    x16 = x_core.astype(np.float16)
    xp = np.zeros((128, BC, D), dtype=np.float16)
    xp[:F0] = x16.transpose(1, 0, 2)
    xt3 = np.zeros((NT, NSC, 128, SW), dtype=np.float16)
    x6 = x16.reshape(NT, NSC, SB, F0, D)
    for j in range(SB):
        xt3[:, :, D * j:D * (j + 1), FW * j:FW * j + F0] = (
            x6[:, :, j].transpose(0, 1, 3, 2))
        xt3[:, :, D * j:D * (j + 1), FW * j + F0] = 1.0
    return xp, np.ascontiguousarray(xt3)


def build(reps: int = 1):
    """Build the per-core Bass module. reps>1 wraps the body in a HW loop
    (wall-clock timing only -- the graded path uses reps=1)."""
    nc = bacc.Bacc("TRN2", target_bir_lowering=False, debug=False,
                   num_devices=NCORES)

    x_h = nc.dram_tensor("xp", [128, BC, D], F16, kind="ExternalInput")
    xt3_h = nc.dram_tensor("xt3", [NT, NSC, 128, SW], F16,
                           kind="ExternalInput")
    vp_h = nc.dram_tensor("vp", [128, NFP], F16, kind="ExternalInput")
    cp_h = nc.dram_tensor("cp", [CW, NCHUNK, H1], F16, kind="ExternalInput")
    w1p_h = nc.dram_tensor("w1p", [H1, F0, H2], F16, kind="ExternalInput")
    id_h = nc.dram_tensor("ident", [128, 128], F16, kind="ExternalInput")
    out_h = nc.dram_tensor("out", [2, 128, BC], F16, kind="ExternalOutput")

    xt3_ap = xt3_h.ap().rearrange("t c p w -> p t c w")  # [128, 8, 4, 320]

    with tile.TileContext(nc) as tc:
        with (
            tc.tile_pool(name="const", bufs=1) as const,
            tc.tile_pool(name="xpool", bufs=3) as xpool,
            tc.tile_pool(name="xtp", bufs=4) as xtp,
            tc.tile_pool(name="ysq", bufs=2) as ysqp,
            tc.tile_pool(name="h1sb", bufs=3) as h1sbp,
            tc.tile_pool(name="h1t", bufs=2) as h1tp,
            tc.tile_pool(name="ps", space="PSUM", bufs=1) as ps,
        ):
            v_sb = const.tile([128, NFP], F16)
            c_sb = const.tile([CW, NCHUNK, H1], F16)
            id_sb = const.tile([128, 128], F16)
            w1_sb = const.tile([H1, F0, H2], F16)
            out1_sb = const.tile([128, BC], F16)
            out2_sb = const.tile([128, BC], F16)
            sall_sb = const.tile([128, BC, FW], F16)   # S^T: [q, b, f|sum]

            def final_half(hf):
                # out2 for b in [hf*128, hf*128+128); out1 half riding along
                out2_ps = ps.tile([128, 2, N], F32, tag="y", bufs=2)
                for f in range(F0):
                    nc.tensor.matmul(out2_ps[:, 0, :128], w1_sb[:, f, :],
                                     sall_sb[:, ts(hf, 128), f],
                                     start=(f == 0), stop=(f == F0 - 1))
                nc.scalar.copy(out2_sb[:, ts(hf, 128)], out2_ps[:, 0, :128])
                nc.scalar.copy(out1_sb[:, ts(hf, 128)],
                               sall_sb[:, ts(hf, 128), F0])
                nc.sync.dma_start(out=out_h.ap()[1, :, ts(hf, 128)],
                                  in_=out2_sb[:, ts(hf, 128)])
                nc.sync.dma_start(out=out_h.ap()[0, :, ts(hf, 128)],
                                  in_=out1_sb[:, ts(hf, 128)])

            def body(_i=None):
                xs, xts, ysqs, h1s, h1sbs, h1ts = {}, {}, {}, {}, {}, {}

                def vpair(r, g):
                    # chunks 2g, 2g+1 (g=3: chunk 6 alone); one batched
                    # ACT square per pair (ACT is the only engine that can
                    # square straight out of PSUM)
                    nchk = 2 if g < 3 else 1
                    y_ps = ps.tile([128, 2, N], F32, tag="y", bufs=2,
                                   name=f"y_{r}_{g}")
                    for u in range(nchk):
                        nc.tensor.matmul(y_ps[:, u, :],
                                         v_sb[:, ts(2 * g + u, CW)],
                                         xs[r][:, :, :], start=True, stop=True)
                    nc.scalar.square(ysqs[r][:, 2 * g:2 * g + nchk, :],
                                     y_ps[:, :nchk, :])

                def cmm(r, j):
                    nc.tensor.matmul(h1s[r][:], c_sb[:, j, :],
                                     ysqs[r][:, j, :],
                                     start=(j == 0), stop=(j == NCHUNK - 1))

                for r in range(NT + 3):
                    if r == 0:
                        nc.sync.dma_start(out=v_sb[:], in_=vp_h.ap())
                    if r < NT:
                        x_sb = xpool.tile([128, BT, D], F16, tag="x", name=f"x_{r}")
                        nc.sync.dma_start(out=x_sb[:],
                                          in_=x_h.ap()[:, ts(r, BT), :])
                        xs[r] = x_sb
                        if r == 0:
                            nc.sync.dma_start(out=c_sb[:], in_=cp_h.ap())
                        xt_sb = xtp.tile([128, NSC, SW], F16, tag="xt", name=f"xt_{r}")
                        nc.sync.dma_start(out=xt_sb[:], in_=xt3_ap[:, r])
                        xts[r] = xt_sb
                        if r == 0:
                            nc.sync.dma_start(out=id_sb[:], in_=id_h.ap())
                        if r == 2:
                            # w1 is only needed for the final contraction
                            nc.sync.dma_start(out=w1_sb[:], in_=w1p_h.ap())
                        ysqs[r] = ysqp.tile([128, NCHUNK, N], F16, tag="ysq",
                                            name=f"ysq_{r}")
                    if 1 <= r < NT + 1:
                        h1s[r - 1] = ps.tile([128, N], F32, tag="h1",
                                             bufs=1, name=f"h1_{r - 1}")

                    # DVE's first op of the round: h1 -> SBUF fp16 copy
                    # (producer C(r-2) stopped at the end of last round)
                    if 2 <= r < NT + 2:
                        t = r - 2
                        h1sbs[t] = h1sbp.tile([128, N], F16, tag="h1sb",
                                              name=f"h1sb_{t}")
                        nc.vector.tensor_copy(out=h1sbs[t][:], in_=h1s[t][:])

                    # ---- PE: V pairs 0,1 | C(r-1) 0..3 | V pair 2 ----
                    if r < NT:
                        vpair(r, 0)
                        vpair(r, 1)
                    if 1 <= r < NT + 1:
                        for j in range(4):
                            cmm(r - 1, j)
                    if r < NT:
                        vpair(r, 2)

                    # ---- S^T matmuls for tile r-3; DVE copies to sall ----
                    if 3 <= r:
                        t = r - 3
                        for u in range(NSC):
                            st_ps = ps.tile([128, 512], F32, tag="aux",
                                            bufs=3, name=f"st_{t}_{u}")
                            nc.tensor.matmul(st_ps[:, :SW], h1ts[t][:, u, :],
                                             xts[t][:, u, :],
                                             start=True, stop=True)
                            b0 = BT * t + SB * u
                            # steady state: DVE; in the tail (no squares
                            # left) alternate ACT/DVE to halve the drip
                            src = st_ps[:, :SW].rearrange(
                                "p (j f) -> p j f", f=FW)
                            dst = sall_sb[:, b0:b0 + SB, :]
                            if t >= 5 and u % 2 == 0:
                                nc.scalar.copy(dst, src)
                            else:
                                nc.vector.tensor_copy(out=dst, in_=src)

                    # ---- transposes for tile r-2; DVE copy last ----
                    if 2 <= r < NT + 2:
                        t = r - 2
                        h1t_ps = ps.tile([128, NSC, 128], F16, tag="aux",
                                         bufs=3, name=f"h1t_ps_{t}")
                        for u in range(NSC):
                            nc.tensor.transpose(h1t_ps[:, u, :],
                                                h1sbs[t][:, ts(u, 128)],
                                                id_sb[:])
                        h1ts[t] = h1tp.tile([128, NSC, 128], F16, tag="h1t",
                                            name=f"h1t_{t}")
                        nc.vector.tensor_copy(out=h1ts[t][:], in_=h1t_ps[:])

                    # ---- PE: V pair 3 | C(r-1) 4..6 ----
                    if r < NT:
                        vpair(r, 3)
                    if 1 <= r < NT + 1:
                        for j in range(4, NCHUNK):
                            cmm(r - 1, j)

                    if r == NT - 1:
                        final_half(0)   # b 0..127: tiles 0..3 are done

                final_half(1)

            if reps == 1:
                body()
            else:
                with tc.For_i(0, reps) as i:
                    body(i)

    nc.compile()
    return nc


_CACHE: dict = {}


def _get_module(reps: int = 1):
    if reps not in _CACHE:
        _CACHE[reps] = build(reps)
    return _CACHE[reps]


def run(input: np.ndarray, W0: np.ndarray, W1: np.ndarray, reps: int = 1):
    nc = _get_module(reps)
    packs = pack_weights(np.asarray(W0), np.asarray(W1))
    x_np = np.asarray(input)
    in_maps = []
    for c in range(NCORES):
        xp, xt3 = pack_x(x_np[c * BC:(c + 1) * BC])
        m = {"xp": xp, "xt3": xt3}
        m.update(packs)
        in_maps.append(m)
    res = run_bass_kernel_spmd(nc, in_maps, core_ids=list(range(NCORES)))
    out = np.empty((B, 256), dtype=np.float32)
    for c in range(NCORES):
        o = res.results[c]["out"]          # [2, 128, 256] fp16
        out[c * BC:(c + 1) * BC, :128] = o[0].T
        out[c * BC:(c + 1) * BC, 128:] = o[1].T
    return out


def kernel(input: np.ndarray, W0: np.ndarray, W1: np.ndarray) -> np.ndarray:
    return run(input, W0, W1, reps=1)
